# revision 17
# baseline (speedup 1.0000x reference)
"""ChemProp message-to-node + MLP kernel for 8 TRN2 NeuronCores.

Strategy (no collectives needed):
  - Host assigns nodes to cores by global degree rank, round-robin, so
    each core receives exactly the edges destined for its nodes and all
    cores see near-identical degree sequences (minimal padding).
  - Edge features are quantized to fp8-e4m3 with per-node ERROR FEEDBACK
    (the quantization residual of each edge is carried into the next
    edge of the same node), so the device-side sum has the error of a
    single quantization step instead of sqrt(deg) steps.  This halves
    the dominant DMA stream vs bf16 while keeping rel-err ~8e-3.
  - Edges are laid out as PAIRS [128, 2, cols]: a DoubleRow fp8 matmul
    against a stacked identity [128, 2, 128] sums both lanes of each
    column into the destination node's PSUM column at 0.5 PE
    cycles/column (the matmul performs the first level of the reduction
    tree for free).  Odd-degree nodes zero-pad their last lane.
  - Node groups of <=512 (one PSUM window).  Within a group, nodes are
    sorted by degree (desc).  Pair-slot q holds edges (2q, 2q+1) of
    every node with deg > 2q, so each slot is a contiguous run of
    columns adding into a prefix of the group's message accumulator.
  - Layout is feature-major ([256, cols] split into 2x128 partitions) so
    the MLP runs without any transposes: hidden^T = W1^T @ cat^T etc.
    MLP in bf16 with f32 PSUM.  PSUM->SBUF copies run on DVE (idle
    otherwise); ReLU on the Act engine; stream DMA on the SP queue,
    rT/weights on gpsimd (SWDGE), output stores on the Act queue.
  - Per-core output slice is returned feature-major (bf16); host
    transposes, un-permutes, casts to f32 and concatenates.
"""

import numpy as np
import ml_dtypes

import concourse.bacc as bacc
import concourse.mybir as mybir
import concourse.tile as tile
from concourse.bass_utils import run_bass_kernel_spmd

NC = 8          # cores
GRP = 512       # nodes per group (one PSUM window = one f32 bank)
CHUNK = 2048    # stream-chunk pair-columns
STREAM_BUFS = 10
MSG_BUFS = 4
SKEW = 0        # groups of seg-sum lead over the MLP (sw pipeline)
PSUM_MSG_BUFS = 2
HID_BUFS = 2
RT_FP8 = True   # r slices in e3m4 (else bf16)
RT_BATCH = 4    # groups per rT load strip
OUT_BATCH = 4   # groups per out store strip

BF16 = mybir.dt.bfloat16
F32 = mybir.dt.float32
FP8E4 = mybir.dt.float8e4
FP8E3 = mybir.dt.float8e3
NP_BF16 = ml_dtypes.bfloat16
NP_E4 = ml_dtypes.float8_e4m3
NP_E3 = ml_dtypes.float8_e3m4


# ----------------------------------------------------------------- host side
def _feedback_quantize(hs_sorted, deg, starts):
    """e4m3-quantize the dst-sorted edge features, carrying each node's
    quantization residual into its next edge (error feedback)."""
    E, F = hs_sorted.shape
    hq = np.zeros((E + 1, F), dtype=NP_E4)      # +1 zero pad row
    maxdeg = int(deg.max()) if E else 0
    carry = np.zeros((deg.shape[0], F), dtype=np.float32)
    for d in range(maxdeg):
        act = np.nonzero(deg > d)[0]
        idx = starts[act] + d
        v = hs_sorted[idx].astype(np.float32) + carry[act]
        q = v.astype(NP_E4)
        hq[idx] = q
        carry[act] = v - q.astype(np.float32)
    return hq


def _preprocess(r, h, nbrs):
    """Build per-core streams/permutations. Returns layout + per-core arrays."""
    n_nodes, Fdim = r.shape
    n_edges = h.shape[0]
    npc = n_nodes // NC
    # small warm-up groups so the first MLP work is ready quickly
    warm = [128, 256]
    body = npc - sum(warm)
    caps = warm + [GRP] * (body // GRP)
    rem = body % GRP
    if rem:
        caps.append(rem)
    grp_lo = np.concatenate([[0], np.cumsum(caps)]).astype(np.int64)
    ngrp = len(caps)

    dst = nbrs[:, 0].astype(np.int64)
    deg_flat = np.bincount(dst, minlength=n_nodes)
    order = np.argsort(dst, kind="stable")          # edges sorted by dest
    starts = np.zeros(n_nodes + 1, dtype=np.int64)
    np.cumsum(deg_flat, out=starts[1:])

    # Node -> (core, position): global degree rank, round-robin over cores,
    # then round-robin over groups within the core; within a group positions
    # are filled in degree-desc order (slot-prefix property).
    rank = np.argsort(-deg_flat, kind="stable")     # rank idx -> global node
    node_ids = np.zeros((NC, npc), dtype=np.int64)  # position -> global node
    deg_sorted = np.zeros((NC, npc), dtype=np.int64)
    for c in range(NC):
        ids_q = rank[c::NC]                          # degree-desc for core c
        fill = [0] * ngrp
        for q in range(npc):
            g = q % ngrp
            while fill[g] == caps[g]:
                g = (g + 1) % ngrp
            pos = int(grp_lo[g]) + fill[g]
            fill[g] += 1
            node_ids[c, pos] = ids_q[q]
        deg_sorted[c] = deg_flat[node_ids[c]]

    # regularized PAIR-slot widths: K2[g][q] = max over cores of #nodes with
    # deg > 2q (those have an edge in pair-slot q).  Slot 0 forced to full
    # group width so every msg column is written by the accumulation group.
    K2 = [None] * ngrp
    slot_off = [None] * ngrp
    off = 0
    for g in range(ngrp):
        lo = int(grp_lo[g])
        hi = int(grp_lo[g + 1])
        w = hi - lo
        degs = deg_sorted[:, lo:hi]                  # [NC, w]
        dmax = max(int(degs.max()), 1)
        npair = (dmax + 1) // 2
        qs = 2 * np.arange(npair)[None, None, :]
        counts = (degs[:, :, None] > qs).sum(1)      # [NC, npair]
        Kg = counts.max(0)
        Kg[0] = w
        offs = off + np.concatenate([[0], np.cumsum(Kg)])
        K2[g] = Kg.astype(np.int64)
        slot_off[g] = offs.astype(np.int64)
        off = int(offs[-1])
    cols = off

    # col -> sorted-edge idx per lane (n_edges = zero pad), per core
    col_sedge = np.full((NC, cols, 2), n_edges, dtype=np.int64)
    for c in range(NC):
        for g in range(ngrp):
            lo = int(grp_lo[g])
            degs_g = deg_sorted[c, lo:int(grp_lo[g + 1])]
            st_g = starts[node_ids[c, lo:int(grp_lo[g + 1])]]
            for q in range(len(K2[g])):
                k0 = int((degs_g > 2 * q).sum())     # lane-0 present
                if k0 == 0:
                    continue
                c0 = int(slot_off[g][q])
                col_sedge[c, c0:c0 + k0, 0] = st_g[:k0] + 2 * q
                k1 = int((degs_g > 2 * q + 1).sum())  # lane-1 present
                if k1:
                    col_sedge[c, c0:c0 + k1, 1] = st_g[:k1] + 2 * q + 1

    return {
        "npc": npc, "ngrp": ngrp, "cols": cols, "F": Fdim,
        "K2": K2, "slot_off": slot_off, "node_ids": node_ids,
        "col_sedge": col_sedge, "grp_lo": grp_lo,
        "deg": deg_flat, "order": order, "starts": starts,
    }


def _build_streams(h, r, lay):
    """Materialize per-core device input arrays."""
    n_edges, Fdim = h.shape
    npc, cols = lay["npc"], lay["cols"]
    fp = Fdim // 128                                 # feature partition-tiles

    hs_sorted = np.ascontiguousarray(h[lay["order"]])
    hq = _feedback_quantize(hs_sorted, lay["deg"], lay["starts"])  # [E+1, F]
    rdt = NP_E3 if RT_FP8 else NP_BF16
    hs, rT = [], []
    for c in range(NC):
        block = hq[lay["col_sedge"][c]]              # [cols, 2, F] fp8
        # -> [fp, 128, 2, cols]
        hs.append(np.ascontiguousarray(
            block.transpose(2, 1, 0)).reshape(fp, 128, 2, cols))
        rc = r[lay["node_ids"][c]].astype(rdt)
        rT.append(np.ascontiguousarray(rc.T).reshape(fp, 128, npc))
    return hs, rT


# --------------------------------------------------------------- device side
def _build_graph(lay, Fdim, H, Fout):
    npc, ngrp, cols = lay["npc"], lay["ngrp"], lay["cols"]
    fp = Fdim // 128          # 2 feature ptiles
    kt_n = (2 * Fdim) // 128  # 4 k-chunks for W1
    ht_n = H // 128           # 4 hidden ptiles
    ot_n = Fout // 128        # 2 output ptiles
    rdt = FP8E3 if RT_FP8 else BF16

    nc = bacc.Bacc(None, target_bir_lowering=False)
    hs_p = nc.declare_dram_parameter("hs", [fp, 128, 2, cols], FP8E4,
                                     isOutput=False)
    idp_p = nc.declare_dram_parameter("idp", [128, 2, 128], FP8E4,
                                      isOutput=False)
    rT_p = nc.declare_dram_parameter("rT", [fp, 128, npc], rdt, isOutput=False)
    w1_p = nc.declare_dram_parameter("W1", [128, kt_n * H], BF16,
                                     isOutput=False)
    w2_p = nc.declare_dram_parameter("W2", [128, ht_n * Fout], BF16,
                                     isOutput=False)
    out_p = nc.declare_dram_parameter("out", [ot_n, 128, npc], BF16,
                                      isOutput=True)

    n_chunks = (cols + CHUNK - 1) // CHUNK

    with tile.TileContext(nc) as tc:
        with (
            tc.tile_pool(name="const", bufs=1) as const_pool,
            tc.tile_pool(name="stream", bufs=STREAM_BUFS) as stream_pool,
            tc.tile_pool(name="msgp", bufs=PSUM_MSG_BUFS, space="PSUM") as msg_psum_pool,
            tc.tile_pool(name="msgb", bufs=MSG_BUFS) as msg_pool,
            tc.tile_pool(name="rb", bufs=3) as r_pool,
            tc.tile_pool(name="mlp1p", bufs=2, space="PSUM") as mlp1_psum_pool,
            tc.tile_pool(name="mlp2p", bufs=2, space="PSUM") as mlp2_psum_pool,
            tc.tile_pool(name="hid", bufs=HID_BUFS) as hid_pool,
            tc.tile_pool(name="osb", bufs=2) as out_pool,
        ):
            # weights + stacked identity resident in SBUF
            idp = const_pool.tile([128, 2, 128], FP8E4, tag="idp")
            nc.sync.dma_start(out=idp[:], in_=idp_p[:, :, :])

            chunk_tiles = [[None] * n_chunks for _ in range(fp)]

            def get_chunk(p, ci, eng=None):
                if chunk_tiles[p][ci] is None:
                    w = min(CHUNK, cols - ci * CHUNK)
                    t = stream_pool.tile([128, 2, w], FP8E4, tag=f"hs{p}",
                                         name="hs_t")
                    (eng or nc.sync).dma_start(
                        out=t[:], in_=hs_p[p, :, :, ci * CHUNK:ci * CHUNK + w])
                    chunk_tiles[p][ci] = t
                return chunk_tiles[p][ci]

            # prime the pipeline: group-0 chunks via SWDGE (its descriptor
            # generation beats the HWDGE path), then r strip 0 and the
            # packed weights, each a single descriptor-set
            get_chunk(0, 0)
            get_chunk(1, 0)
            rb_strip = []
            for p in range(fp):
                t = r_pool.tile([128, int(lay["grp_lo"][min(RT_BATCH, ngrp)])],
                                rdt, tag=f"rb{p}", name=f"rb{p}")
                nc.gpsimd.dma_start(out=t[:], in_=rT_p[p, :, 0:t.shape[-1]])
                rb_strip.append(t)
            rb_base = 0
            w1_all = const_pool.tile([128, kt_n * H], BF16, tag="w1")
            nc.gpsimd.dma_start(out=w1_all[:], in_=w1_p[:, :])
            w2_all = const_pool.tile([128, ht_n * Fout], BF16, tag="w2")
            nc.gpsimd.dma_start(out=w2_all[:], in_=w2_p[:, :])
            w1_sb = [w1_all[:, k * H:(k + 1) * H] for k in range(kt_n)]
            w2_sb = [w2_all[:, k * Fout:(k + 1) * Fout] for k in range(ht_n)]

            def emit_seg(gi):
                g = gi
                lo = int(lay["grp_lo"][g])
                w_g = int(lay["grp_lo"][g + 1]) - lo

                # pair-slot spans split on chunk boundaries
                pieces = []   # (chunk, src_off, dst_off, len)
                for q in range(len(lay["K2"][g])):
                    c0 = int(lay["slot_off"][g][q])
                    k = int(lay["K2"][g][q])
                    s = c0
                    while s < c0 + k:
                        ci = s // CHUNK
                        e = min(c0 + k, (ci + 1) * CHUNK)
                        pieces.append((ci, s - ci * CHUNK, s - c0, e - s))
                        s = e

                msgb = []
                for p in range(fp):
                    ps = msg_psum_pool.tile([128, w_g], F32, space="PSUM",
                                            tag=f"mp{p}", name="mps")
                    for i, (ci, o0, dj, ln) in enumerate(pieces):
                        src = get_chunk(p, ci)
                        # exactly ONE start=True per PSUM window; untouched
                        # columns first-touch via has_written on start=False
                        nc.tensor.matmul(
                            out=ps[:, dj:dj + ln],
                            lhsT=idp[:],
                            rhs=src[:, :, o0:o0 + ln],
                            start=(i == 0),
                            stop=(i == len(pieces) - 1),
                            perf_mode=mybir.MatmulPerfMode.DoubleRow,
                            skip_group_check=True,
                        )
                    mb = msg_pool.tile([128, w_g], BF16, tag=f"mb{p}",
                                       name="mbs")
                    nc.vector.tensor_copy(out=mb[:], in_=ps[:])
                    msgb.append(mb)
                return msgb

            state = {"rb_strip": rb_strip, "rb_base": rb_base,
                     "ob_strips": None, "ob_lo": 0}

            def emit_mlp(gi, msgb):
                g = gi
                lo = int(lay["grp_lo"][g])
                w_g = int(lay["grp_lo"][g + 1]) - lo

                # r slice; loaded in RT_BATCH-group strips
                if gi % RT_BATCH == 0 and gi > 0:
                    b_lo = lo
                    b_hi = int(lay["grp_lo"][min(g + RT_BATCH, ngrp)])
                    state["rb_strip"] = []
                    for p in range(fp):
                        t = r_pool.tile([128, b_hi - b_lo], rdt, tag=f"rb{p}",
                                        name="rbs")
                        nc.gpsimd.dma_start(out=t[:], in_=rT_p[p, :, b_lo:b_hi])
                        state["rb_strip"].append(t)
                    state["rb_base"] = b_lo
                rb_base_ = state["rb_base"]
                rb = [t[:, lo - rb_base_:lo - rb_base_ + w_g]
                      for t in state["rb_strip"]]
                cat = rb + msgb  # k-chunk order matches W1 rows

                # MLP: hidden^T = relu(W1^T @ cat^T)
                hid = []
                for ht in range(ht_n):
                    ps = mlp1_psum_pool.tile([128, w_g], F32, space="PSUM",
                                             tag="mlp1", name="m1ps")
                    for k in range(kt_n):
                        nc.tensor.matmul(
                            out=ps[:],
                            lhsT=w1_sb[k][:, ht * 128:(ht + 1) * 128],
                            rhs=cat[k][:],
                            start=(k == 0), stop=(k == kt_n - 1))
                    hb = hid_pool.tile([128, w_g], BF16, tag=f"h{ht}",
                                       name="hbs")
                    nc.scalar.activation(
                        hb[:], ps[:], mybir.ActivationFunctionType.Relu)
                    hid.append(hb)

                # out^T = W2^T @ hidden^T
                for ot in range(ot_n):
                    ps = mlp2_psum_pool.tile([128, w_g], F32, space="PSUM",
                                             tag="mlp2", name="m2ps")
                    for k in range(ht_n):
                        nc.tensor.matmul(
                            out=ps[:],
                            lhsT=w2_sb[k][:, ot * 128:(ot + 1) * 128],
                            rhs=hid[k][:],
                            start=(k == 0), stop=(k == ht_n - 1))
                    if gi % OUT_BATCH == 0 and ot == 0:
                        state["ob_lo"] = lo
                        ob_hi = int(lay["grp_lo"][min(g + OUT_BATCH, ngrp)])
                        state["ob_strips"] = [
                            out_pool.tile([128, ob_hi - state["ob_lo"]], BF16,
                                          tag=f"o{o}", name="obs")
                            for o in range(ot_n)]
                    ob_lo = state["ob_lo"]
                    ob_t = state["ob_strips"][ot]
                    nc.scalar.activation(
                        ob_t[:, lo - ob_lo:lo - ob_lo + w_g],
                        ps[:], mybir.ActivationFunctionType.Copy)
                    if gi % OUT_BATCH == OUT_BATCH - 1 or gi == ngrp - 1:
                        nc.scalar.dma_start(
                            out=out_p[ot, :, ob_lo:ob_lo + ob_t.shape[1]],
                            in_=ob_t[:])

            msgq = {}
            for gi in range(ngrp + SKEW):
                if gi < ngrp:
                    msgq[gi] = emit_seg(gi)
                if gi >= SKEW:
                    emit_mlp(gi - SKEW, msgq.pop(gi - SKEW))

    nc.finalize()
    return nc


# ----------------------------------------------------------------- interface
def prepare(r, h, nbrs, W1, W2):
    """Preprocess inputs + build the Bass graph. Returns everything needed
    to run and to assemble the output."""
    r = np.asarray(r, dtype=np.float32)
    h = np.asarray(h, dtype=np.float32)
    nbrs = np.asarray(nbrs)
    W1 = np.asarray(W1, dtype=np.float32)
    W2 = np.asarray(W2, dtype=np.float32)

    n_nodes, Fdim = r.shape
    H = W1.shape[1]
    Fout = W2.shape[1]

    lay = _preprocess(r, h, nbrs)
    hs, rT = _build_streams(h, r, lay)
    w1d = np.ascontiguousarray(
        W1.astype(NP_BF16).reshape(-1, 128, H).transpose(1, 0, 2).reshape(
            128, -1))
    w2d = np.ascontiguousarray(
        W2.astype(NP_BF16).reshape(-1, 128, Fout).transpose(1, 0, 2).reshape(
            128, -1))
    idp = np.zeros((128, 2, 128), dtype=NP_E4)
    for k in range(2):
        idp[np.arange(128), k, np.arange(128)] = 1.0

    nc = _build_graph(lay, Fdim, H, Fout)
    in_maps = [
        {"hs": hs[c], "rT": rT[c], "W1": w1d, "W2": w2d, "idp": idp}
        for c in range(NC)
    ]
    return {"nc": nc, "in_maps": in_maps, "lay": lay,
            "n_nodes": n_nodes, "Fout": Fout}


def assemble(prep, results):
    lay = prep["lay"]
    n_nodes, Fout = prep["n_nodes"], prep["Fout"]
    npc = lay["npc"]
    out = np.zeros((n_nodes, Fout), dtype=np.float32)
    for c in range(NC):
        o = np.asarray(results[c]["out"]).reshape(Fout, npc)
        out[lay["node_ids"][c]] = o.T.astype(np.float32)
    return out


def kernel(r, h, nbrs, W1, W2):
    prep = prepare(r, h, nbrs, W1, W2)
    res = run_bass_kernel_spmd(prep["nc"], prep["in_maps"],
                               core_ids=list(range(NC)))
    return assemble(prep, res.results)
